# revision 85
# baseline (speedup 1.0000x reference)
"""AttnDecoder kernel for 8 trn2 NeuronCores.

Math: the reference's additive attention has NO nonlinearity between W1/W2/w3,
so scores[b,t,s] = enc[b,s]@ue + dec[b,t]@ud + const. Softmax over s cancels
the t-dependent terms exactly -> attn (and ctx) are t-independent:
    ue  = W1[:, :H].T @ (W2.T @ w3[0])
    attn[b, :] = softmax(enc[b] @ ue);  ctx[b] = attn[b] @ enc[b]
Device work = 2-layer LSTM (replicated on all 8 cores) + vocab-sharded output
projection (4000 rows/core). No collectives.

Orientation: everything runs transposed — activations are [feature, batch]
column vectors and the weight matrix is the matmul's STATIONARY operand
(lhsT [K=128 indim, M=128 gates]), the moving operand is the h/x column block
[128, 8]. Gate chunks land in psum as [128 gates, 8 batch] blocks, the cell
update runs on [128, 32] column tiles, and h is produced directly in the
layout the next matmul consumes — no transposes anywhere.
"""

import numpy as np
import ml_dtypes

B, T, S = 8, 64, 128
V, E, H = 32000, 512, 512
NCORES = 8
VS = V // NCORES       # 4000 vocab rows per core
NV = 32                # vocab chunks per core
MV = VS // NV          # 125 rows per chunk
KC = 4                 # 128-col chunks per 512 dim
GC = 16                # gate chunks (2048 / 128)

_BF16 = ml_dtypes.bfloat16
_DEBUG = False


def _gate_reorder(W):
    """PyTorch gate order i,f,g,o -> our order i,f,o,g (rows of [4H, D]).
    Unless OPTS['use_tanh'], g-gate rows are scaled x2 so that
    tanh(z) = 2*sigmoid(2z) - 1 needs a single Sigmoid over all gates."""
    Hh = W.shape[0] // 4
    gs = 2.0 if (OPTS["tanh_affine"] or not OPTS["use_tanh"]) else 1.0
    return np.concatenate([W[:Hh], W[Hh:2 * Hh],
                           gs * W[2 * Hh:3 * Hh], W[3 * Hh:]], 0)


def _wT_blocks(W):
    """W [2048, 512] (gates i,f,o,g x indim) -> [128, 4*16*128] bf16 sbuf
    layout: block (k, m) at cols (k*16+m)*128; lhsT[p, m'] = W[128m+m', 128k+p].
    """
    out = np.zeros((128, KC * GC * 128), np.float32)
    for k in range(KC):
        for m in range(GC):
            blk = W[128 * m:128 * (m + 1), 128 * k:128 * (k + 1)]  # [M, K]
            out[:, (k * GC + m) * 128:(k * GC + m + 1) * 128] = blk.T
    return out


def _colT(v):
    """[B, D] -> [128, (D/128)*8]: col 8k+b = v[b, 128k+p]."""
    D = v.shape[1]
    kc = D // 128
    o = np.zeros((128, kc * 8), np.float32)
    for k in range(kc):
        o[:, 8 * k:8 * (k + 1) + 0] = v[:, 128 * k:128 * (k + 1)].T
    return o


OPTS = {
    "do_proj": True,       # emit the output projection
    "memset_act": False,   # L1 psum memset on Act instead of DVE
    "use_tanh": True,      # direct Tanh activation instead of 2*sig(2x)-1
    "m1_pool": False,      # m1 multiply on gpsimd
    "dma_zero": False,     # zero psum tiles via DMA instead of engine memset
    "pe_zero": True,       # zero psum tiles via rank-1 matmul on PE
    "psg_bufs": 2,         # psum buffering depth for gate tiles
    "tanh_affine": True,   # all gates via ONE Tanh(scale=0.5); h~=2h folded
    "proj_interleave": True,  # run proj t<32 half during LSTM steps 33..63
}


def _build_nc(has_bias):
    import concourse.bass as bass
    import concourse.bacc as bacc
    import concourse.mybir as mybir
    import concourse.tile as tile

    f32 = mybir.dt.float32
    bf16 = mybir.dt.bfloat16
    AF = mybir.ActivationFunctionType
    OP = mybir.AluOpType

    nc = bacc.Bacc(None, target_bir_lowering=False)
    d = {}
    # weights: [128, 4*16*128] blocks (k, m)
    for nm in ("Wx0", "Wh0", "Wx1", "Wh1"):
        d[nm] = nc.dram_tensor(nm, [128, KC * GC * 128], bf16, kind="ExternalInput")
    d["xT"] = nc.dram_tensor("xT", [128, T * 32], bf16, kind="ExternalInput")
    d["h0a"] = nc.dram_tensor("h0a", [128, 32], bf16, kind="ExternalInput")
    d["h0b"] = nc.dram_tensor("h0b", [128, 32], bf16, kind="ExternalInput")
    d["c0"] = nc.dram_tensor("c0", [128, 64], f32, kind="ExternalInput")
    if has_bias:
        d["bg"] = nc.dram_tensor("bg", [1, 2 * GC * 128], bf16, kind="ExternalInput")
    if OPTS["dma_zero"]:
        d["zeros"] = nc.dram_tensor("zeros", [128, 128], f32,
                                    kind="ExternalInput")
    d["Wo"] = nc.dram_tensor("Wo", [128, NV * KC * MV], bf16, kind="ExternalInput")
    out_d = nc.dram_tensor("out", [VS, T * 8], bf16, kind="ExternalOutput")
    if _DEBUG:
        dbg0 = nc.dram_tensor("dbg0", [128, (T + 1) * 32], bf16,
                              kind="ExternalOutput")
        dbg1 = nc.dram_tensor("dbg1", [128, (T + 1) * 32], bf16,
                              kind="ExternalOutput")
        dbgp = nc.dram_tensor("dbgp", [128, 256], f32, kind="ExternalOutput")

    with tile.TileContext(nc) as tc:
        with (
            tc.tile_pool(name="const", bufs=1) as cp,
            tc.tile_pool(name="work", bufs=4) as wp,
            tc.tile_pool(name="psg", bufs=OPTS["psg_bufs"], space="PSUM") as ppg,
            tc.tile_pool(name="pso", bufs=4, space="PSUM") as ppo,
        ):
            Wx0 = cp.tile([128, KC * GC * 128], bf16, tag="Wx0")
            Wh0 = cp.tile([128, KC * GC * 128], bf16, tag="Wh0")
            Wx1 = cp.tile([128, KC * GC * 128], bf16, tag="Wx1")
            Wh1 = cp.tile([128, KC * GC * 128], bf16, tag="Wh1")
            Wos = cp.tile([128, NV * KC * MV], bf16, tag="Wos")
            xTs = cp.tile([128, T * 32], bf16, tag="xTs")
            hist0 = cp.tile([128, (T + 1) * 32], bf16, tag="hist0")
            hist1 = cp.tile([128, (T + 1) * 32], bf16, tag="hist1")
            c_sb = cp.tile([128, 64], f32, tag="c_sb")
            if has_bias:
                bgs = cp.tile([1, 2 * GC * 128], bf16, tag="bgs")
                ones8 = cp.tile([1, 8], bf16, tag="ones8")
            if OPTS["pe_zero"]:
                zrow = cp.tile([1, 128], bf16, tag="zrow")
                orow = cp.tile([1, 128], bf16, tag="orow")
                nc.vector.memset(zrow[:], 0.0)
                nc.vector.memset(orow[:], 1.0)

            # Layer-0 weights + inputs first (needed immediately)
            for k in range(KC):
                de = nc.sync if k % 2 == 0 else nc.gpsimd
                de.dma_start(Wx0[:, k * 2048:(k + 1) * 2048],
                             d["Wx0"][:, k * 2048:(k + 1) * 2048])
            nc.sync.dma_start(xTs[:], d["xT"][:])
            nc.sync.dma_start(hist0[:, 0:32], d["h0a"][:])
            nc.sync.dma_start(hist1[:, 0:32], d["h0b"][:])
            nc.sync.dma_start(c_sb[:], d["c0"][:])
            for k in range(KC):
                de = nc.sync if k % 2 == 0 else nc.gpsimd
                de.dma_start(Wh0[:, k * 2048:(k + 1) * 2048],
                             d["Wh0"][:, k * 2048:(k + 1) * 2048])
            if has_bias:
                nc.sync.dma_start(bgs[:], d["bg"][:])
                nc.vector.memset(ones8[:], 1.0)
            for k in range(KC):
                de = nc.sync if k % 2 == 0 else nc.gpsimd
                de.dma_start(Wx1[:, k * 2048:(k + 1) * 2048],
                             d["Wx1"][:, k * 2048:(k + 1) * 2048])
            for k in range(KC):
                de = nc.sync if k % 2 == 0 else nc.gpsimd
                de.dma_start(Wh1[:, k * 2048:(k + 1) * 2048],
                             d["Wh1"][:, k * 2048:(k + 1) * 2048])
            for r in range(8):
                w = NV * KC * MV // 8
                de = nc.sync if r % 2 == 0 else nc.gpsimd
                de.dma_start(Wos[:, r * w:(r + 1) * w],
                             d["Wo"][:, r * w:(r + 1) * w])

            Wx = [Wx0, Wx1]
            Wh = [Wh0, Wh1]
            hist = [hist0, hist1]

            def lstm_matmuls(layer, t):
                # gates.T in psum: [128, 128], col 8m+b; m = 4*G + k,
                # G in (i, f, o, g)
                pg = ppg.tile([128, 128], f32, tag=f"pg{layer}",
                              name=f"pg{layer}_{t}")
                xsrc = xTs if layer == 0 else hist0
                xoff = 32 * t if layer == 0 else 32 * (t + 1)
                # NO start=True anywhere: interleaved accumulation groups in
                # one psum 512B window clobber each other's pending start
                # writes (scheduler interleaves regions). Prefill with memset
                # and pure-accumulate instead -- safe in any order.
                if OPTS["pe_zero"]:
                    nc.tensor.matmul(pg[:], zrow[0:1, :], orow[0:1, :],
                                     start=True, stop=False,
                                     skip_group_check=True)
                elif OPTS["dma_zero"]:
                    nc.sync.dma_start(pg[:], d["zeros"][:])
                elif OPTS["memset_act"] and layer == 1:
                    nc.scalar.memzero(pg[:])
                else:
                    nc.vector.memset(pg[:], 0.0)
                if has_bias:
                    for m in range(GC):
                        nc.tensor.matmul(
                            pg[:, 8 * m:8 * m + 8],
                            bgs[0:1, (layer * GC + m) * 128:
                                (layer * GC + m + 1) * 128],
                            ones8[0:1, :], start=False, stop=False,
                            skip_group_check=True)
                for m in range(GC):
                    for k in range(KC):
                        nc.tensor.matmul(
                            pg[:, 8 * m:8 * m + 8],
                            Wx[layer][:, (k * GC + m) * 128:
                                      (k * GC + m + 1) * 128],
                            xsrc[:, xoff + 8 * k:xoff + 8 * k + 8],
                            start=False, stop=False, skip_group_check=True)
                for m in range(GC):
                    for k in range(KC):
                        nc.tensor.matmul(
                            pg[:, 8 * m:8 * m + 8],
                            Wh[layer][:, (k * GC + m) * 128:
                                      (k * GC + m + 1) * 128],
                            hist[layer][:, 32 * t + 8 * k:32 * t + 8 * k + 8],
                            start=False, stop=(k == KC - 1),
                            skip_group_check=True)
                if _DEBUG and t == 0:
                    dpg = cp.tile([128, 128], f32, tag=f"dpg{layer}")
                    nc.vector.tensor_copy(dpg[:], pg[:])
                    nc.sync.dma_start(dbgp[:, 128 * layer:128 * (layer + 1)],
                                      dpg[:])
                return pg

            # granular chain pieces; issue order = scheduler priority
            st = {}
            AFF = OPTS["tanh_affine"]

            def p_acts(layer, pg):
                sg = wp.tile([128, 128], f32, tag=f"sg{layer}", name=f"sg{layer}")
                if AFF:
                    # gate order i,f,g,o: t = tanh(gate/2) for i,f,o
                    # (sig(x)=(t+1)/2), tanh(g) for g (weights carry x2).
                    # o is needed only by hmul much later -> separate op so
                    # the cell chain starts off a smaller, earlier tanh.
                    nc.scalar.activation(sg[:, 0:96], pg[:, 0:96],
                                         AF.Tanh, scale=0.5)
                    nc.scalar.activation(sg[:, 96:128], pg[:, 96:128],
                                         AF.Tanh, scale=0.5)
                else:
                    nc.scalar.activation(sg[:, 0:96], pg[:, 0:96], AF.Sigmoid)
                    nc.scalar.activation(sg[:, 96:128], pg[:, 96:128], AF.Tanh)
                st[layer] = sg

            def p_cell(layer):
                sg = st[layer]
                cs = c_sb[:, 32 * layer:32 * (layer + 1)]
                m1 = wp.tile([128, 32], f32, tag=f"m1{layer}", name=f"m1{layer}")
                m2 = wp.tile([128, 32], f32, tag=f"m2{layer}", name=f"m2{layer}")
                if AFF:
                    # u = (t_f+1)*c + (t_i+1)*t_g = 2*c_new
                    u = wp.tile([128, 32], f32, tag=f"u{layer}", name=f"u{layer}")
                    if OPTS["m1_pool"]:
                        # Pool ISA lacks TensorScalarPtr; use mult then add
                        mm = wp.tile([128, 32], f32, tag=f"mm{layer}",
                                     name=f"mm{layer}")
                        nc.gpsimd.tensor_mul(mm[:], sg[:, 32:64], cs)
                        nc.gpsimd.tensor_add(m1[:], mm[:], cs)
                    else:
                        nc.vector.scalar_tensor_tensor(
                            m1[:], sg[:, 32:64], 1.0, cs, OP.add, OP.mult)
                    nc.vector.scalar_tensor_tensor(
                        m2[:], sg[:, 0:32], 1.0, sg[:, 64:96],
                        OP.add, OP.mult)
                    nc.vector.tensor_add(u[:], m1[:], m2[:])
                    st[(layer, "u")] = u
                else:
                    nc.vector.tensor_mul(m1[:], sg[:, 32:64], cs)
                    nc.vector.tensor_mul(m2[:], sg[:, 96:128], sg[:, 0:32])
                    nc.vector.tensor_add(cs, m1[:], m2[:])

            def p_tanhc(layer):
                th = wp.tile([128, 32], f32, tag=f"th{layer}", name=f"th{layer}")
                if AFF:
                    nc.scalar.activation(th[:], st[(layer, "u")][:],
                                         AF.Tanh, scale=0.5)
                else:
                    cs = c_sb[:, 32 * layer:32 * (layer + 1)]
                    nc.scalar.activation(th[:], cs, AF.Tanh)
                st[(layer, "th")] = th

            def p_hmul(layer, t):
                sg = st[layer]
                th = st[(layer, "th")]
                hdst = hist[layer][:, 32 * (t + 1):32 * (t + 2)]
                if AFF:
                    # h~ = 2h = (t_o+1)*tanh(c); consumers' weights carry 0.5
                    nc.vector.scalar_tensor_tensor(
                        hdst, sg[:, 96:128], 1.0, th[:], OP.add, OP.mult)
                else:
                    nc.vector.tensor_mul(hdst, th[:], sg[:, 64:96])

            def p_chalf(layer):
                # c = u/2; consumed only by NEXT step's m1 -- off the
                # critical cycle, issued after hmul
                cs = c_sb[:, 32 * layer:32 * (layer + 1)]
                nc.vector.tensor_scalar_mul(cs, st[(layer, "u")][:], 0.5)

            # dec.T chunk k as moving operand: hist1 cols 32(t+1)+8k+b
            dv = hist1.rearrange("p (s c) -> p s c", c=32)

            obgrp = {}

            def proj_chunk(n, h, eng):
                # out.T [vocab chunk 125, (t in half h, b) 256]; copies land
                # in a 4-chunk group buffer, DMA'd once per group (the DGE
                # stage serializes DMAs at ~625ns each)
                po = ppo.tile([128, T * 4], f32, tag="po", name=f"po{n}_{h}")
                for k in range(KC):
                    nc.tensor.matmul(
                        po[0:MV, :],
                        Wos[:, (n * KC + k) * MV:(n * KC + k + 1) * MV],
                        dv[:, 1 + 32 * h:33 + 32 * h, 8 * k:8 * k + 8],
                        start=(k == 0), stop=(k == KC - 1))
                ob = wp.tile([128, T * 4], bf16, tag=f"ob{n % 2}",
                             name=f"ob{n}_{h}")
                if eng == 0:
                    nc.vector.tensor_copy(ob[0:MV, :], po[0:MV, :])
                else:
                    nc.scalar.copy(ob[0:MV, :], po[0:MV, :])
                dma_eng = nc.sync if n % 2 == 0 else nc.gpsimd
                dma_eng.dma_start(
                    out_d[n * MV:(n + 1) * MV, 256 * h:256 * (h + 1)],
                    ob[0:MV, :])

            def proj_group_dma(g, h):
                pass

            for t in range(T):
                pg0 = lstm_matmuls(0, t)
                pg1 = lstm_matmuls(1, t - 1) if t >= 1 else None
                p_acts(0, pg0)
                p_cell(0)
                p_tanhc(0)
                p_hmul(0, t)
                if pg1 is not None:
                    p_acts(1, pg1)
                    p_cell(1)
                if AFF:
                    p_chalf(0)
                if pg1 is not None:
                    p_tanhc(1)
                    p_hmul(1, t - 1)
                    if AFF:
                        p_chalf(1)
                if OPTS["proj_interleave"] and t >= 44 and t < 60:
                    n = 2 * (t - 44)
                    proj_chunk(n, 0, 1)
                    proj_chunk(n + 1, 0, 0)
                    if n % 4 == 3:
                        proj_group_dma(n // 4, 0)
            pgl = lstm_matmuls(1, T - 1)
            p_acts(1, pgl)
            p_cell(1)
            p_tanhc(1)
            p_hmul(1, T - 1)
            if AFF:
                p_chalf(1)

            tail = [] if OPTS["proj_interleave"] else \
                [(n, 0) for n in range(NV)]
            tail += [(n, 1) for n in range(NV)]
            for i, (n, h) in enumerate(tail):
                proj_chunk(n, h, i % 2)
                if n % 4 == 3:
                    proj_group_dma(n // 4, h)

            if _DEBUG:
                nc.sync.dma_start(dbg0[:], hist0[:])
                nc.sync.dma_start(dbg1[:], hist1[:])
    nc.finalize()
    return nc


_NC_CACHE = {}


def _get_nc(has_bias):
    if has_bias not in _NC_CACHE:
        _NC_CACHE[has_bias] = _build_nc(has_bias)
    return _NC_CACHE[has_bias]


def _host_inputs(input_ids, enc_output, h0, c0, emb, Wih0, Whh0, bih0, bhh0,
                 Wih1, Whh1, bih1, bhh1, W1, b1, W2, b2, w3, b3, Wout, bout):
    f32 = np.float32
    x = np.asarray(emb, f32)[np.asarray(input_ids).astype(np.int64)]  # [B,T,E]
    # xT: [128, 32t + 8k + b]
    xr = x.transpose(2, 0, 1).reshape(KC, 128, B, T)      # [k, p, b, t]
    xT = xr.transpose(1, 3, 0, 2).reshape(128, T * 32)    # [p, t, k, b]

    c0r = np.asarray(c0, f32)
    c_sp = np.concatenate([_colT(c0r[0]), _colT(c0r[1])], 1)  # [128, 64]

    # collapsed attention (exact in real arithmetic; see module docstring)
    u = np.asarray(W2, f32).T @ np.asarray(w3, f32)[0]
    ue = np.asarray(W1, f32)[:, :H].T @ u
    sc = np.asarray(enc_output, f32) @ ue                  # [B,S]
    sc = sc - sc.max(-1, keepdims=True)
    a = np.exp(sc)
    a /= a.sum(-1, keepdims=True)
    ctxh = np.einsum('bs,bsh->bh', a, np.asarray(enc_output, f32))  # [B,H]

    bsum0 = np.asarray(bih0, f32) + np.asarray(bhh0, f32)
    bsum1 = np.asarray(bih1, f32) + np.asarray(bhh1, f32)
    has_bias = bool(np.any(bsum0) or np.any(bsum1))

    # tanh_affine: hist holds h~ = 2h, so weights consuming hist carry x0.5;
    # initial h states are doubled to match.
    hs = 0.5 if OPTS["tanh_affine"] else 1.0
    hs0 = 2.0 if OPTS["tanh_affine"] else 1.0
    base = {
        "xT": xT.astype(_BF16),
        **({"zeros": np.zeros((128, 128), f32)} if OPTS["dma_zero"] else {}),
        "h0a": (hs0 * _colT(np.asarray(h0, f32)[0])).astype(_BF16),
        "h0b": (hs0 * _colT(np.asarray(h0, f32)[1])).astype(_BF16),
        "c0": c_sp,
        "Wx0": _wT_blocks(_gate_reorder(np.asarray(Wih0, f32))).astype(_BF16),
        "Wh0": _wT_blocks(_gate_reorder(hs * np.asarray(Whh0, f32))).astype(_BF16),
        "Wx1": _wT_blocks(_gate_reorder(hs * np.asarray(Wih1, f32))).astype(_BF16),
        "Wh1": _wT_blocks(_gate_reorder(hs * np.asarray(Whh1, f32))).astype(_BF16),
    }
    if has_bias:
        bg = np.concatenate([_gate_reorder(bsum0.reshape(4 * H, 1)),
                             _gate_reorder(bsum1.reshape(4 * H, 1))], 0)
        base["bg"] = bg.reshape(1, 2 * GC * 128).astype(_BF16)

    Wo_full = np.asarray(Wout, f32)                        # [V, 2H]
    # bias2[b, v] = ctx[b] @ Wout[v, 512:] + bout[v]; added on host (f32)
    bias2 = ctxh @ Wo_full[:, H:].T + np.asarray(bout, f32)[None, :]  # [8, V]
    maps = []
    for c in range(NCORES):
        sh = hs * Wo_full[c * VS:(c + 1) * VS, :H]         # [VS, 512] dec part
        wo = np.zeros((128, NV * KC * MV), np.float32)
        for n in range(NV):
            for k in range(KC):
                blk = sh[n * MV:(n + 1) * MV, 128 * k:128 * (k + 1)]  # [M, K]
                wo[:, (n * KC + k) * MV:(n * KC + k + 1) * MV] = blk.T
        m = dict(base)
        m["Wo"] = wo.astype(_BF16)
        maps.append(m)
    return maps, has_bias, bias2


def kernel(**inputs):
    from concourse.bass_utils import run_bass_kernel_spmd
    maps, has_bias, bias2 = _host_inputs(**inputs)
    nc = _get_nc(has_bias)
    res = run_bass_kernel_spmd(nc, maps, list(range(NCORES))).results
    full = np.zeros((B, T, V), np.float32)
    for c in range(NCORES):
        o = np.asarray(res[c]["out"], np.float32).reshape(VS, T, B)
        full[:, :, c * VS:(c + 1) * VS] = o.transpose(2, 1, 0)
    full += bias2[:, None, :]
    return full
